# revision 1
# baseline (speedup 1.0000x reference)
"""LookAheadMask kernel for Trainium2.

out[b, r, c] = 1.0 if c > r else x[b, r, c], for x of shape (8, 4096, 4096) f32.

Sharding: batch dim across 8 NeuronCores (data parallel, no communication).

Per-core plan (matrix is S x S, S=4096, row-blocks of P=128), raw bass
(explicit engines + semaphores; the Tile drain would exceed walrus's
sync-wait-slot limit with this many independent DMAs):

  - strictly-lower region (cols < block start): 31 direct DRAM->DRAM copies
  - strictly-upper region (cols >= block end): 31 DMAs from an SBUF ones
    tile (no HBM read for that half)
  - the 32 diagonal 128x128 blocks: one 3D-strided gather DMA into SBUF
    [128, 32*128], one gpsimd affine_select (keep x where
    row >= col-within-block, else 1.0), one scatter back.

A single HWDGE ring executes queued DMAs one at a time (profiled: slice
durations sum to the whole span), so the 62 bulk DMAs are split round-robin
across three descriptor paths that run concurrently: SP ring (sync), ACT
ring (scalar), and SWDGE (gpsimd). Row-block i's copy (i*64KB) and ones
((31-i)*64KB) pair to ~2MB, so assigning pairs round-robin balances bytes.

HBM traffic/core: ~33 MiB read + 64 MiB write vs 128 MiB naive.
"""

import numpy as np

from concourse import bass, mybir
from concourse.bass_utils import run_bass_kernel_spmd

S = 4096
P = 128
NB = S // P  # 32
N_CORES = 8

_cached_nc = None


def _build():
    global _cached_nc
    if _cached_nc is not None:
        return _cached_nc

    nc = bass.Bass()
    x = nc.dram_tensor("x", [S, S], mybir.dt.float32, kind="ExternalInput")
    out = nc.dram_tensor("out", [S, S], mybir.dt.float32, kind="ExternalOutput")

    # Diagonal-block view: [row-in-block(128), block(32), col-in-block(128)],
    # block b starts at element offset b*(P*S + P). Strides in elements.
    diag_pairs = [[S, P], [P * S + P, NB], [1, P]]
    # Gather window: W cols per diag block ending at its right edge, so
    # descriptors are W*4 bytes instead of 512 (the 512B-descriptor gather
    # profiled at 152us for 2MB). Blocks 1..31 in one DMA; block 0's window
    # would start before the tensor, so it gets its own 128-col load.
    W = 256

    # 62 bulk DMAs all HWDGE (dsem): 47 on the SP ring, 15 on the ACT ring
    # (issued after the ACT ring's cheap wide-window diag gather)

    def bulk(eng, blocks, ones):
        """Emit copy then ones DMAs for the given row-blocks on one engine."""
        for i in blocks:
            r0 = i * P
            if i > 0:
                eng.dma_start(
                    out=out[r0 : r0 + P, 0:r0], in_=x[r0 : r0 + P, 0:r0]
                ).then_inc(dsem, 16)
        eng.wait_ge(msem, 1)
        for i in blocks:
            r0 = i * P
            if i < NB - 1:
                w = S - r0 - P
                eng.dma_start(
                    out=out[r0 : r0 + P, r0 + P : S], in_=ones[:, :w]
                ).then_inc(dsem, 16)

    with (
        nc.Block() as block,
        nc.semaphore("dsem") as dsem,  # bulk DMA completions (HWDGE rings)
        nc.semaphore("gsem") as gsem,  # diag gather done
        nc.semaphore("ssem") as ssem,  # diag scatter done
        nc.semaphore("msem") as msem,  # ones memset done
        nc.semaphore("asem") as asem,  # affine_select done
        nc.sbuf_tensor("ones", [P, S], mybir.dt.float32) as ones,
        nc.sbuf_tensor("diag_in2", [P, NB * W], mybir.dt.float32) as diag_in2,
        nc.sbuf_tensor("diag_out", [P, S], mybir.dt.float32) as diag_out,
    ):

        @block.vector
        def _(vector: bass.BassVectorEngine):
            vector.memset(ones[:, :], 1.0).then_inc(msem, 1)

        @block.scalar
        def _(scalar: bass.BassEngine):
            scalar.dma_start(
                out=bass.AP(diag_in2, W, [[NB * W, P], [W, NB - 1], [1, W]]),
                in_=bass.AP(x, (P * S + P) + P - W, [[S, P], [P * S + P, NB - 1], [1, W]]),
            ).then_inc(gsem, 16)
            scalar.dma_start(
                out=bass.AP(diag_in2, W - P, [[NB * W, P], [1, P]]),
                in_=x[0:P, 0:P],
            ).then_inc(gsem, 16)
            bulk(scalar, range(3, NB, 4), ones)
            scalar.wait_ge(asem, 1)
            scalar.dma_start(
                out=bass.AP(out, 0, diag_pairs), in_=diag_out[:, :]
            ).then_inc(ssem, 16)

        @block.gpsimd
        def _(gpsimd: bass.BassGpSimd):
            gpsimd.wait_ge(gsem, 32)
            # iota[p, c] = p - (c % 128); keep x where >= 0 (at/below diag).
            # Input reads the last 128 cols of each W-wide gathered block.
            gpsimd.affine_select(
                out=diag_out[:, :],
                in_=bass.AP(diag_in2, W - P, [[NB * W, P], [W, NB], [1, P]]),
                pattern=[[0, NB], [-1, P]],
                base=0,
                channel_multiplier=1,
                compare_op=mybir.AluOpType.is_ge,
                fill=1.0,
            ).then_inc(asem, 1)

        @block.sync
        def _(sync: bass.BassEngine):
            bulk(sync, [i for i in range(NB) if i % 4 != 3], ones)
            sync.wait_ge(dsem, 16 * 62)
            sync.wait_ge(ssem, 16)

    _cached_nc = nc
    return nc


def _run(x_full: np.ndarray, trace: bool = False):
    nc = _build()
    x_full = np.asarray(x_full, dtype=np.float32)
    in_maps = [{"x": x_full[i]} for i in range(N_CORES)]
    res = run_bass_kernel_spmd(nc, in_maps, list(range(N_CORES)), trace=trace)
    out = np.stack([res.results[i]["out"] for i in range(N_CORES)], axis=0)
    return out, res


def kernel(x: np.ndarray) -> np.ndarray:
    out, _ = _run(x, trace=False)
    return out



# revision 3
# speedup vs baseline: 1.0838x; 1.0838x over previous
"""LookAheadMask kernel for Trainium2.

out[b, r, c] = 1.0 if c > r else x[b, r, c], for x of shape (8, 4096, 4096) f32.

Sharding: batch dim across 8 NeuronCores (data parallel, no communication).

The op is an in-place masked_fill: out == x everywhere except the strictly
upper triangle, which is constant 1.0.  The PJRT launch path donates a
host-staged buffer as the kernel's output tensor (the stock runner stages
zeros and kernels rely on that zero-init); here we stage x itself, so the
device kernel only writes the masked region instead of first copying the
31 MiB lower triangle through HBM twice (DRAM->DRAM read+write).  Per-core
HBM traffic drops from ~99 MiB to ~33 MiB, which matters because the
measured baseline was HBM-bandwidth-bound (both HWDGE queues >90% busy at
a combined ~324 GB/s, right at the per-core HBM share).

Device-side plan per core (S=4096, P=128):

  - strict-upper staircase at 256-block granularity: a quad-tree of
    same-size square rectangles; all rects of size w sit at constant
    stride 2w(S+1), so each level is ONE 3D-strided DMA from an SBUF
    ones tile (5 DMAs, 30 MiB, descriptors 1-8 KiB).
  - the within-256-block triangles: out[r, r+1 : r+257] = 1 for
    r < 3840 -- a "shifted band" write.  Everything right of the
    diagonal is 1, so over-writing into the staircase region is
    harmless, and no gather of x is needed at all (the baseline's
    gather/scatter of the diagonal blocks cost ~120us of ring time at
    48-54 GB/s due to sub-1KiB descriptors).  Two DMAs with AP stride
    S+1, 1 KiB descriptors.
  - bottom-right 256x256 corner (rows 3840+, where the band would
    overrun the row end): one 256 KiB gather -> gpsimd affine_select
    (keep x at/below diagonal, 1.0 above) -> one scatter.

DMAs are spread over the three descriptor paths (SP ring via sync, ACT
ring via scalar, SWDGE via gpsimd) balancing estimated ring occupancy.
"""

import glob
import os
import tempfile

import numpy as np

from concourse import bass, mybir

S = 4096
P = 128
N_CORES = 8

_cached_nc = None


def _build():
    global _cached_nc
    if _cached_nc is not None:
        return _cached_nc

    nc = bass.Bass()
    out = nc.dram_tensor("out", [S, S], mybir.dt.float32, kind="ExternalOutput")

    # ones tile: 128 x 16384 f32 (8 MiB).  Largest single ones DMA below
    # needs 2M source elements (the 2048-level halves and the 1024 level).
    F = 16384

    with (
        nc.Block() as block,
        nc.semaphore("dsem") as dsem,  # bulk ones/band DMA completions
        nc.semaphore("gsem") as gsem,  # corner gather done
        nc.semaphore("ssem") as ssem,  # corner scatter done
        nc.semaphore("msem") as msem,  # ones memset halves done (2 total)
        nc.semaphore("asem") as asem,  # corner affine_select done
        nc.sbuf_tensor("ones", [P, F], mybir.dt.float32) as ones,
        nc.sbuf_tensor("corner_in", [P, 512], mybir.dt.float32) as corner_in,
        nc.sbuf_tensor("corner_out", [P, 512], mybir.dt.float32) as corner_out,
    ):
        # Quad-tree staircase level: width w, 2048//w rects, rect k at
        # rows [2wk, 2wk+w), cols [2wk+w, 2wk+2w)  => offset w, stride
        # 2w(S+1).  Source: first c*w*w/128 cols of the ones tile (the
        # element ORDER does not matter -- every element is 1.0).
        def ones_level(eng, w):
            c = 2048 // w
            eng.dma_start(
                out=bass.AP(out, w, [[2 * w * (S + 1), c], [S, w], [1, w]]),
                in_=ones[:, : c * w * w // P],
            ).then_inc(dsem, 16)

        # 2048-level split into two 1024-row halves (source tile holds 2M
        # elements, the full level is 4M).
        def ones_2048_half(eng, half):
            eng.dma_start(
                out=bass.AP(out, half * 1024 * S + 2048, [[S, 1024], [1, 2048]]),
                in_=ones[:, :F],
            ).then_inc(dsem, 16)

        # Shifted-band write: out[r, r+1 : r+257] = 1 for nrows rows
        # starting at row r0 (covers the in-block triangles; overlap with
        # the staircase writes the same 1.0 value).
        def band(eng, r0, nrows):
            eng.dma_start(
                out=bass.AP(out, r0 * (S + 1) + 1, [[S + 1, nrows], [1, 256]]),
                in_=ones[:, : nrows * 256 // P],
            ).then_inc(dsem, 16)

        # Corner [3840:4096) x [3840:4096) as [p, block(2), 256]
        corner_dram = bass.AP(
            out, 3840 * S + 3840, [[S, P], [P * S, 2], [1, 256]]
        )
        corner_sb = [[512, P], [256, 2], [1, 256]]

        @block.vector
        def _(vector: bass.BassVectorEngine):
            vector.memset(ones[:, : F // 2], 1.0).then_inc(msem, 1)
            vector.memset(ones[:, F // 2 :], 1.0).then_inc(msem, 1)

        @block.scalar
        def _(scalar: bass.BassEngine):
            scalar.dma_start(
                out=bass.AP(corner_in, 0, corner_sb), in_=corner_dram
            ).then_inc(gsem, 16)
            scalar.wait_ge(msem, 1)
            band(scalar, 1920, 1920)
            scalar.wait_ge(msem, 2)
            ones_level(scalar, 1024)
            scalar.wait_ge(asem, 1)
            scalar.dma_start(
                out=corner_dram, in_=bass.AP(corner_out, 0, corner_sb)
            ).then_inc(ssem, 16)

        @block.gpsimd
        def _(gpsimd: bass.BassGpSimd):
            gpsimd.wait_ge(msem, 1)
            ones_level(gpsimd, 256)
            gpsimd.wait_ge(msem, 2)
            ones_2048_half(gpsimd, 1)
            gpsimd.wait_ge(gsem, 16)
            # iota(p, i, c2) = p + 128*i - c2 ; keep x where >= 0 (at or
            # below the diagonal), else fill 1.0.  Block i=0 is rows
            # 3840..3967 (diag at c2=p), block i=1 rows 3968..4095 (diag
            # at c2=p+128); both windows span cols 3840..4095.
            gpsimd.affine_select(
                out=corner_out[:, :],
                in_=bass.AP(corner_in, 0, corner_sb),
                pattern=[[P, 2], [-1, 256]],
                base=0,
                channel_multiplier=1,
                compare_op=mybir.AluOpType.is_ge,
                fill=1.0,
            ).then_inc(asem, 1)

        @block.sync
        def _(sync: bass.BassEngine):
            sync.wait_ge(msem, 1)
            ones_level(sync, 512)
            band(sync, 0, 1920)
            sync.wait_ge(msem, 2)
            ones_2048_half(sync, 0)
            sync.wait_ge(dsem, 16 * 7)
            sync.wait_ge(ssem, 16)

    _cached_nc = nc
    return nc


def _sharded_fn(nc):
    """Build the 8-core PJRT launcher with the output buffer donated.

    Mirrors concourse.bass2jax.run_bass_via_pjrt's multi-core path, except
    the donated output staging buffer is caller-provided (we stage x, the
    in-place masked_fill source) instead of zeros.
    """
    import jax
    from concourse import bass2jax as b2j

    b2j.install_neuronx_cc_hook()

    partition_name = nc.partition_id_tensor.name if nc.partition_id_tensor else None
    in_names: list = []
    out_names: list = []
    out_avals: list = []
    for alloc in nc.m.functions[0].allocations:
        if not isinstance(alloc, mybir.MemoryLocationSet):
            continue
        name = alloc.memorylocations[0].name
        if alloc.kind == "ExternalInput":
            if name != partition_name:
                in_names.append(name)
        elif alloc.kind == "ExternalOutput":
            assert alloc.tensor_shape is not None and alloc.dtype is not None
            out_names.append(name)
            out_avals.append(
                jax.core.ShapedArray(tuple(alloc.tensor_shape), mybir.dt.np(alloc.dtype))
            )
    assert in_names == [] and out_names == ["out"], (in_names, out_names)
    all_in_names = tuple(in_names + out_names + ([partition_name] if partition_name else []))

    def _body(out_buf):
        operands = [out_buf]
        if partition_name is not None:
            operands.append(b2j.partition_id_tensor())
        outs = b2j._bass_exec_p.bind(
            *operands,
            out_avals=tuple(out_avals),
            in_names=all_in_names,
            out_names=tuple(out_names),
            lowering_input_output_aliases=(),
            sim_require_finite=True,
            sim_require_nnan=True,
            nc=nc,
        )
        return tuple(outs)

    devices = jax.devices()[:N_CORES]
    assert len(devices) == N_CORES, f"need {N_CORES} devices, got {len(devices)}"
    mesh = b2j.Mesh(np.asarray(devices), ("core",))
    spec = (b2j.PartitionSpec("core"),)
    return jax.jit(
        b2j.shard_map(_body, mesh=mesh, in_specs=spec, out_specs=spec, check_rep=False),
        donate_argnums=(0,),
        keep_unused=True,
    )


_cached_fn = None


def _run(x_full: np.ndarray, trace: bool = False):
    global _cached_fn
    nc = _build()
    if _cached_fn is None:
        _cached_fn = _sharded_fn(nc)

    x_full = np.ascontiguousarray(np.asarray(x_full, dtype=np.float32))
    staged = x_full.reshape(N_CORES * S, S)

    if not trace:
        out = _cached_fn(staged)[0]
        return np.asarray(out).reshape(N_CORES, S, S), None

    # Profiling path (test harness only): capture core 0's NTFF via the
    # axon hook and run the stock NTFF -> perfetto pipeline.
    from antenv.axon_hooks import get_axon_ntff_profile_hook
    import gauge.profiler
    from concourse import bass_utils
    from concourse._compat import FishPath

    hook = get_axon_ntff_profile_hook()
    neff_dir = tempfile.mkdtemp()
    with hook(neff_dir, [0]):
        out = _cached_fn(staged)[0]
    result = np.asarray(out).reshape(N_CORES, S, S)

    if not glob.glob(os.path.join(neff_dir, "*_body*.ntff")):
        return result, bass_utils.BassKernelResults(
            results=[], instructions_and_trace=None, profile_json=None,
            exec_time_ns=None,
        )
    sharepath = bass_utils.upload_artifacts(neff_dir)
    profile = gauge.profiler.Profile(
        profile_path=FishPath(neff_dir),
        kernel_dev_mode=True,
        profile_on_exit=False,
        bass_kernel=nc.m,
        offline_processing=True,
        fname="*_body*",
        metadata={"artifacts_path": sharepath},
    )
    res = bass_utils._process_ntff_profile(
        profile, neff_dir, nc, list(range(N_CORES)), [0], False, {}, False
    )
    return result, res.as_bass_kernel_results([])


def kernel(x: np.ndarray) -> np.ndarray:
    out, _ = _run(x, trace=False)
    return out


# revision 4
# speedup vs baseline: 1.8659x; 1.7216x over previous
"""LookAheadMask kernel for Trainium2.

out[b, r, c] = 1.0 if c > r else x[b, r, c], for x of shape (8, 4096, 4096) f32.

Sharding: batch dim across 8 NeuronCores (data parallel, no communication).

The op is an in-place masked_fill: out == x everywhere except the strictly
upper triangle, which is constant 1.0.  The PJRT launch path donates a
host-staged buffer as the kernel's output tensor (the stock runner stages
zeros and kernels rely on that zero-init); here we stage x itself, so the
device kernel only writes the masked region instead of first copying the
31 MiB lower triangle through HBM twice (DRAM->DRAM read+write).  Per-core
HBM traffic drops from ~99 MiB to ~33 MiB, which matters because the
measured baseline was HBM-bandwidth-bound (both HWDGE queues >90% busy at
a combined ~324 GB/s, right at the per-core HBM share).

Device-side plan per core (S=4096, P=128):

  - strict-upper staircase at 256-block granularity: a quad-tree of
    same-size square rectangles; all rects of size w sit at constant
    stride 2w(S+1), so each level is ONE 3D-strided DMA from an SBUF
    ones tile (5 DMAs, 30 MiB, descriptors 1-8 KiB).
  - the within-256-block triangles: out[r, r+1 : r+257] = 1 for
    r < 3840 -- a "shifted band" write.  Everything right of the
    diagonal is 1, so over-writing into the staircase region is
    harmless, and no gather of x is needed at all (the baseline's
    gather/scatter of the diagonal blocks cost ~120us of ring time at
    48-54 GB/s due to sub-1KiB descriptors).  Two DMAs with AP stride
    S+1, 1 KiB descriptors.
  - bottom-right 256x256 corner (rows 3840+, where the band would
    overrun the row end): one 256 KiB gather -> gpsimd affine_select
    (keep x at/below diagonal, 1.0 above) -> one scatter.

DMAs are spread over the three descriptor paths (SP ring via sync, ACT
ring via scalar, SWDGE via gpsimd) balancing estimated ring occupancy.
"""

import glob
import os
import tempfile

import numpy as np

from concourse import bass, mybir

S = 4096
P = 128
N_CORES = 8

_cached_nc = None


def _build():
    global _cached_nc
    if _cached_nc is not None:
        return _cached_nc

    nc = bass.Bass()
    out = nc.dram_tensor("out", [S, S], mybir.dt.float32, kind="ExternalOutput")

    # ones tile: 128 x 8192 f32 (4 MiB) -- every DMA below sources at most
    # 1M elements from it (element order is irrelevant: all are 1.0).
    F = 8192

    with (
        nc.Block() as block,
        nc.semaphore("dsem") as dsem,  # bulk ones/band DMA completions
        nc.semaphore("gsem") as gsem,  # corner gather done
        nc.semaphore("ssem") as ssem,  # corner scatter done
        nc.semaphore("msem") as msem,  # ones memset done
        nc.sbuf_tensor("ones", [P, F], mybir.dt.float32) as ones,
        nc.sbuf_tensor("corner_in", [P, P], mybir.dt.float32) as corner_in,
        nc.sbuf_tensor("corner_out", [P, P], mybir.dt.float32) as corner_out,
    ):
        # Quad-tree staircase level: width w, 2048//w rects, rect k at
        # rows [2wk, 2wk+w), cols [2wk+w, 2wk+2w)  => offset w, stride
        # 2w(S+1).
        def ones_level(eng, w):
            c = 2048 // w
            eng.dma_start(
                out=bass.AP(out, w, [[2 * w * (S + 1), c], [S, w], [1, w]]),
                in_=ones[:, : c * w * w // P],
            ).then_inc(dsem, 16)

        # One quarter (512 rows) of the 2048-level rect rows[r0:r0+512] x
        # cols[2048:4096]; 8 KiB descriptors.
        def ones_2048_quarter(eng, r0):
            eng.dma_start(
                out=bass.AP(out, r0 * S + 2048, [[S, 512], [1, 2048]]),
                in_=ones[:, :F],
            ).then_inc(dsem, 16)

        # One rect (1024x1024) of the 1024-level; 4 KiB descriptors.
        def ones_1024_rect(eng, k):
            eng.dma_start(
                out=bass.AP(out, 1024 + k * 2048 * (S + 1), [[S, 1024], [1, 1024]]),
                in_=ones[:, :F],
            ).then_inc(dsem, 16)

        # Corner: the last 128x128 diagonal block, rows/cols [3968:4096).
        corner_dram = bass.AP(out, 3968 * S + 3968, [[S, P], [1, P]])

        @block.vector
        def _(vector: bass.BassVectorEngine):
            vector.memset(ones[:, :], 1.0).then_inc(msem, 1)

        @block.sync
        def _(sync: bass.BassEngine):
            sync.wait_ge(msem, 1)
            ones_1024_rect(sync, 0)
            ones_2048_quarter(sync, 0)
            ones_2048_quarter(sync, 512)
            sync.wait_ge(dsem, 16 * 10)
            sync.wait_ge(ssem, 16)

        @block.scalar
        def _(scalar: bass.BassEngine):
            scalar.wait_ge(msem, 1)
            ones_1024_rect(scalar, 1)
            ones_2048_quarter(scalar, 1024)
            ones_2048_quarter(scalar, 1536)

        @block.gpsimd
        def _(gpsimd: bass.BassGpSimd):
            # Corner gather first: nothing else touches rows/cols 3968+.
            gpsimd.dma_start(out=corner_in[:, :], in_=corner_dram).then_inc(gsem, 16)
            gpsimd.wait_ge(msem, 1)
            # Small-descriptor work lives on SWDGE: it packs sub-4KiB
            # descriptors into 4 KiB packets (HWDGE does not), and all
            # queues are packet-rate-bound at ~30 ns/packet.
            ones_level(gpsimd, 512)
            ones_level(gpsimd, 256)
            ones_level(gpsimd, 128)
            # Shifted-band write covering the in-block triangles:
            # out[r, r+1 : r+129) = 1 for rows 0..3967; right of the
            # diagonal everything is 1, so spilling into the staircase
            # region is a harmless same-value overlap.
            gpsimd.dma_start(
                out=bass.AP(out, 1, [[S + 1, 3968], [1, P]]),
                in_=ones[:, :3968],
            ).then_inc(dsem, 16)
            gpsimd.wait_ge(gsem, 16)
            # iota(p, c) = p - c; keep x where >= 0 (at/below diagonal).
            gpsimd.affine_select(
                out=corner_out[:, :],
                in_=corner_in[:, :],
                pattern=[[-1, P]],
                base=0,
                channel_multiplier=1,
                compare_op=mybir.AluOpType.is_ge,
                fill=1.0,
            )
            gpsimd.dma_start(out=corner_dram, in_=corner_out[:, :]).then_inc(ssem, 16)

    _cached_nc = nc
    return nc


def _sharded_fn(nc):
    """Build the 8-core PJRT launcher with the output buffer donated.

    Mirrors concourse.bass2jax.run_bass_via_pjrt's multi-core path, except
    the donated output staging buffer is caller-provided (we stage x, the
    in-place masked_fill source) instead of zeros.
    """
    import jax
    from concourse import bass2jax as b2j

    b2j.install_neuronx_cc_hook()

    partition_name = nc.partition_id_tensor.name if nc.partition_id_tensor else None
    in_names: list = []
    out_names: list = []
    out_avals: list = []
    for alloc in nc.m.functions[0].allocations:
        if not isinstance(alloc, mybir.MemoryLocationSet):
            continue
        name = alloc.memorylocations[0].name
        if alloc.kind == "ExternalInput":
            if name != partition_name:
                in_names.append(name)
        elif alloc.kind == "ExternalOutput":
            assert alloc.tensor_shape is not None and alloc.dtype is not None
            out_names.append(name)
            out_avals.append(
                jax.core.ShapedArray(tuple(alloc.tensor_shape), mybir.dt.np(alloc.dtype))
            )
    assert in_names == [] and out_names == ["out"], (in_names, out_names)
    all_in_names = tuple(in_names + out_names + ([partition_name] if partition_name else []))

    def _body(out_buf):
        operands = [out_buf]
        if partition_name is not None:
            operands.append(b2j.partition_id_tensor())
        outs = b2j._bass_exec_p.bind(
            *operands,
            out_avals=tuple(out_avals),
            in_names=all_in_names,
            out_names=tuple(out_names),
            lowering_input_output_aliases=(),
            sim_require_finite=True,
            sim_require_nnan=True,
            nc=nc,
        )
        return tuple(outs)

    devices = jax.devices()[:N_CORES]
    assert len(devices) == N_CORES, f"need {N_CORES} devices, got {len(devices)}"
    mesh = b2j.Mesh(np.asarray(devices), ("core",))
    spec = (b2j.PartitionSpec("core"),)
    return jax.jit(
        b2j.shard_map(_body, mesh=mesh, in_specs=spec, out_specs=spec, check_rep=False),
        donate_argnums=(0,),
        keep_unused=True,
    )


_cached_fn = None


def _run(x_full: np.ndarray, trace: bool = False):
    global _cached_fn
    nc = _build()
    if _cached_fn is None:
        _cached_fn = _sharded_fn(nc)

    x_full = np.ascontiguousarray(np.asarray(x_full, dtype=np.float32))
    staged = x_full.reshape(N_CORES * S, S)

    if not trace:
        out = _cached_fn(staged)[0]
        return np.asarray(out).reshape(N_CORES, S, S), None

    # Profiling path (test harness only): capture core 0's NTFF via the
    # axon hook and run the stock NTFF -> perfetto pipeline.
    from antenv.axon_hooks import get_axon_ntff_profile_hook
    import gauge.profiler
    from concourse import bass_utils
    from concourse._compat import FishPath

    hook = get_axon_ntff_profile_hook()
    neff_dir = tempfile.mkdtemp()
    with hook(neff_dir, [0]):
        out = _cached_fn(staged)[0]
    result = np.asarray(out).reshape(N_CORES, S, S)

    if not glob.glob(os.path.join(neff_dir, "*_body*.ntff")):
        return result, bass_utils.BassKernelResults(
            results=[], instructions_and_trace=None, profile_json=None,
            exec_time_ns=None,
        )
    sharepath = bass_utils.upload_artifacts(neff_dir)
    profile = gauge.profiler.Profile(
        profile_path=FishPath(neff_dir),
        kernel_dev_mode=True,
        profile_on_exit=False,
        bass_kernel=nc.m,
        offline_processing=True,
        fname="*_body*",
        metadata={"artifacts_path": sharepath},
    )
    res = bass_utils._process_ntff_profile(
        profile, neff_dir, nc, list(range(N_CORES)), [0], False, {}, False
    )
    return result, res.as_bass_kernel_results([])


def kernel(x: np.ndarray) -> np.ndarray:
    out, _ = _run(x, trace=False)
    return out


# revision 9
# speedup vs baseline: 2.1905x; 1.1740x over previous
"""LookAheadMask kernel for Trainium2.

out[b, r, c] = 1.0 if c > r else x[b, r, c], for x of shape (8, 4096, 4096) f32.

Sharding: batch dim across 8 NeuronCores (data parallel, no communication).

The op is an in-place masked_fill: out == x everywhere except the strictly
upper triangle, which is constant 1.0.  The PJRT launch path donates a
host-staged buffer as the kernel's output tensor (the stock runner stages
zeros and kernels rely on that zero-init); here we stage x itself, so the
device kernel only writes the masked region instead of first copying the
31 MiB lower triangle through HBM twice (DRAM->DRAM read+write).  Per-core
HBM traffic drops from ~99 MiB to ~33 MiB, which matters because the
measured baseline was HBM-bandwidth-bound (both HWDGE queues >90% busy at
a combined ~324 GB/s, right at the per-core HBM share).

Device-side plan per core (S=4096, P=128):

  - strict-upper staircase at 256-block granularity: a quad-tree of
    same-size square rectangles; all rects of size w sit at constant
    stride 2w(S+1), so each level is ONE 3D-strided DMA from an SBUF
    ones tile (5 DMAs, 30 MiB, descriptors 1-8 KiB).
  - the within-256-block triangles: out[r, r+1 : r+257] = 1 for
    r < 3840 -- a "shifted band" write.  Everything right of the
    diagonal is 1, so over-writing into the staircase region is
    harmless, and no gather of x is needed at all (the baseline's
    gather/scatter of the diagonal blocks cost ~120us of ring time at
    48-54 GB/s due to sub-1KiB descriptors).  Two DMAs with AP stride
    S+1, 1 KiB descriptors.
  - bottom-right 256x256 corner (rows 3840+, where the band would
    overrun the row end): one 256 KiB gather -> gpsimd affine_select
    (keep x at/below diagonal, 1.0 above) -> one scatter.

DMAs are spread over the three descriptor paths (SP ring via sync, ACT
ring via scalar, SWDGE via gpsimd) balancing estimated ring occupancy.
"""

import glob
import os
import tempfile

import numpy as np

from concourse import bass, mybir

S = 4096
P = 128
N_CORES = 8

_cached_nc = None


def _build():
    global _cached_nc
    if _cached_nc is not None:
        return _cached_nc

    nc = bass.Bass()
    out = nc.dram_tensor("out", [S, S], mybir.dt.float32, kind="ExternalOutput")

    # ones tile: 128 x 4096 f32 (2 MiB) -- every DMA below sources at most
    # 512K elements from it (element order is irrelevant: all are 1.0).
    F = 4096

    with (
        nc.Block() as block,
        nc.semaphore("dsem") as dsem,  # bulk ones/band DMA completions
        nc.semaphore("gsem") as gsem,  # corner gather done
        nc.semaphore("ssem") as ssem,  # corner scatter done
        nc.semaphore("msem") as msem,  # ones memset done
        nc.sbuf_tensor("ones", [P, F], mybir.dt.float32) as ones,
        nc.sbuf_tensor("corner_in", [P, P], mybir.dt.float32) as corner_in,
        nc.sbuf_tensor("corner_out", [P, P], mybir.dt.float32) as corner_out,
    ):
        # Quad-tree staircase level (width w, 2048//w rects, rect k at rows
        # [2wk, 2wk+w) x cols [2wk+w, 2wk+2w)), restricted to rects
        # [k0, k0+c).  All DMAs source <= 512K elements of the ones tile.
        def ones_level(eng, w, k0, c):
            return eng.dma_start(
                out=bass.AP(
                    out, w + k0 * 2 * w * (S + 1), [[2 * w * (S + 1), c], [S, w], [1, w]]
                ),
                in_=ones[:, : c * w * w // P],
            ).then_inc(dsem, 16)

        # 512-row slice of the 2048-level rect (rows[r0:r0+512] x
        # cols[2048:4096] halved into 256-row pieces for the 512K source
        # cap); 8 KiB descriptors.
        def ones_2048_quarter(eng, r0):
            for r in (r0, r0 + 256):
                eng.dma_start(
                    out=bass.AP(out, r * S + 2048, [[S, 256], [1, 2048]]),
                    in_=ones[:, :F],
                ).then_inc(dsem, 16)

        # Half (512 rows) of a 1024-level rect; 4 KiB descriptors.
        def ones_1024_half(eng, k, r0):
            eng.dma_start(
                out=bass.AP(
                    out, 1024 + k * 2048 * (S + 1) + r0 * S, [[S, 512], [1, 1024]]
                ),
                in_=ones[:, :F],
            ).then_inc(dsem, 16)

        # Corner: the last 128x128 diagonal block, rows/cols [3968:4096).
        corner_dram = bass.AP(out, 3968 * S + 3968, [[S, P], [1, P]])

        @block.vector
        def _(vector: bass.BassVectorEngine):
            vector.memset(ones[:, :], 1.0).then_inc(msem, 1)

        @block.sync
        def _(sync: bass.BassEngine):
            sync.wait_ge(msem, 1)
            ones_1024_half(sync, 0, 0)
            ones_1024_half(sync, 0, 512)
            ones_2048_quarter(sync, 0)
            ones_2048_quarter(sync, 512)
            ones_level(sync, 512, 0, 1)
            sync.wait_ge(dsem, 16 * 18)
            sync.wait_ge(ssem, 16)

        @block.scalar
        def _(scalar: bass.BassEngine):
            scalar.wait_ge(msem, 1)
            ones_1024_half(scalar, 1, 0)
            ones_1024_half(scalar, 1, 512)
            ones_2048_quarter(scalar, 1024)
            ones_2048_quarter(scalar, 1536)
            ones_level(scalar, 512, 3, 1)

        @block.gpsimd
        def _(gpsimd: bass.BassGpSimd):
            # Corner gather first: nothing else touches rows/cols 3968+.
            gpsimd.dma_start(out=corner_in[:, :], in_=corner_dram).then_inc(gsem, 16)
            gpsimd.wait_ge(msem, 1)
            # Sub-2KiB-descriptor work lives on SWDGE: it packs small
            # descriptors into 4 KiB packets (HWDGE does not), and all
            # queues are packet-rate-bound at ~30-40 ns/packet.
            ones_level(gpsimd, 512, 1, 2)
            ones_level(gpsimd, 256, 0, 8)
            ones_level(gpsimd, 128, 0, 16)
            # Shifted-band write covering the in-block triangles:
            # out[r, r+1 : r+129) = 1 for rows 0..3967; right of the
            # diagonal everything is 1, so spilling into the staircase
            # region is a harmless same-value overlap.
            gpsimd.dma_start(
                out=bass.AP(out, 1, [[S + 1, 3968], [1, P]]),
                in_=ones[:, :3968],
            ).then_inc(dsem, 16)
            gpsimd.wait_ge(gsem, 16)
            # iota(p, c) = p - c; keep x where >= 0 (at/below diagonal).
            gpsimd.affine_select(
                out=corner_out[:, :],
                in_=corner_in[:, :],
                pattern=[[-1, P]],
                base=0,
                channel_multiplier=1,
                compare_op=mybir.AluOpType.is_ge,
                fill=1.0,
            )
            gpsimd.dma_start(out=corner_dram, in_=corner_out[:, :]).then_inc(ssem, 16)

    _cached_nc = nc
    return nc


def _sharded_fn(nc):
    """Build the 8-core PJRT launcher with the output buffer donated.

    Mirrors concourse.bass2jax.run_bass_via_pjrt's multi-core path, except
    the donated output staging buffer is caller-provided (we stage x, the
    in-place masked_fill source) instead of zeros.
    """
    import jax
    from concourse import bass2jax as b2j

    b2j.install_neuronx_cc_hook()

    partition_name = nc.partition_id_tensor.name if nc.partition_id_tensor else None
    in_names: list = []
    out_names: list = []
    out_avals: list = []
    for alloc in nc.m.functions[0].allocations:
        if not isinstance(alloc, mybir.MemoryLocationSet):
            continue
        name = alloc.memorylocations[0].name
        if alloc.kind == "ExternalInput":
            if name != partition_name:
                in_names.append(name)
        elif alloc.kind == "ExternalOutput":
            assert alloc.tensor_shape is not None and alloc.dtype is not None
            out_names.append(name)
            out_avals.append(
                jax.core.ShapedArray(tuple(alloc.tensor_shape), mybir.dt.np(alloc.dtype))
            )
    assert in_names == [] and out_names == ["out"], (in_names, out_names)
    all_in_names = tuple(in_names + out_names + ([partition_name] if partition_name else []))

    def _body(out_buf):
        operands = [out_buf]
        if partition_name is not None:
            operands.append(b2j.partition_id_tensor())
        outs = b2j._bass_exec_p.bind(
            *operands,
            out_avals=tuple(out_avals),
            in_names=all_in_names,
            out_names=tuple(out_names),
            lowering_input_output_aliases=(),
            sim_require_finite=True,
            sim_require_nnan=True,
            nc=nc,
        )
        return tuple(outs)

    devices = jax.devices()[:N_CORES]
    assert len(devices) == N_CORES, f"need {N_CORES} devices, got {len(devices)}"
    mesh = b2j.Mesh(np.asarray(devices), ("core",))
    spec = (b2j.PartitionSpec("core"),)
    return jax.jit(
        b2j.shard_map(_body, mesh=mesh, in_specs=spec, out_specs=spec, check_rep=False),
        donate_argnums=(0,),
        keep_unused=True,
    )


_cached_fn = None


def _run(x_full: np.ndarray, trace: bool = False):
    global _cached_fn
    nc = _build()
    if _cached_fn is None:
        _cached_fn = _sharded_fn(nc)

    x_full = np.ascontiguousarray(np.asarray(x_full, dtype=np.float32))
    staged = x_full.reshape(N_CORES * S, S)

    if not trace:
        out = _cached_fn(staged)[0]
        return np.asarray(out).reshape(N_CORES, S, S), None

    # Profiling path (test harness only): capture core 0's NTFF via the
    # axon hook and run the stock NTFF -> perfetto pipeline.
    from antenv.axon_hooks import get_axon_ntff_profile_hook
    import gauge.profiler
    from concourse import bass_utils
    from concourse._compat import FishPath

    hook = get_axon_ntff_profile_hook()
    neff_dir = tempfile.mkdtemp()
    with hook(neff_dir, [0]):
        out = _cached_fn(staged)[0]
    result = np.asarray(out).reshape(N_CORES, S, S)

    if not glob.glob(os.path.join(neff_dir, "*_body*.ntff")):
        return result, bass_utils.BassKernelResults(
            results=[], instructions_and_trace=None, profile_json=None,
            exec_time_ns=None,
        )
    sharepath = bass_utils.upload_artifacts(neff_dir)
    profile = gauge.profiler.Profile(
        profile_path=FishPath(neff_dir),
        kernel_dev_mode=True,
        profile_on_exit=False,
        bass_kernel=nc.m,
        offline_processing=True,
        fname="*_body*",
        metadata={"artifacts_path": sharepath},
    )
    res = bass_utils._process_ntff_profile(
        profile, neff_dir, nc, list(range(N_CORES)), [0], False, {}, False
    )
    return result, res.as_bass_kernel_results([])


def kernel(x: np.ndarray) -> np.ndarray:
    out, _ = _run(x, trace=False)
    return out


# revision 11
# speedup vs baseline: 2.5110x; 1.1463x over previous
"""LookAheadMask kernel for Trainium2.

out[b, r, c] = 1.0 if c > r else x[b, r, c], for x of shape (8, 4096, 4096) f32.

Sharding: batch dim across 8 NeuronCores (data parallel, no communication).

The op is an in-place masked_fill: out == x everywhere except the strictly
upper triangle, which is constant 1.0.  The PJRT launch path donates a
host-staged buffer as the kernel's output tensor (the stock runner stages
zeros and kernels rely on that zero-init); here we stage x itself, so the
device kernel only writes the masked region instead of first copying the
31 MiB lower triangle through HBM twice (DRAM->DRAM read+write).  Per-core
HBM traffic drops from ~99 MiB to ~33 MiB, which matters because the
measured baseline was HBM-bandwidth-bound (both HWDGE queues >90% busy at
a combined ~324 GB/s, right at the per-core HBM share).

Device-side plan per core (S=4096, P=128):

  - strict-upper staircase at 256-block granularity: a quad-tree of
    same-size square rectangles; all rects of size w sit at constant
    stride 2w(S+1), so each level is ONE 3D-strided DMA from an SBUF
    ones tile (5 DMAs, 30 MiB, descriptors 1-8 KiB).
  - the within-256-block triangles: out[r, r+1 : r+257] = 1 for
    r < 3840 -- a "shifted band" write.  Everything right of the
    diagonal is 1, so over-writing into the staircase region is
    harmless, and no gather of x is needed at all (the baseline's
    gather/scatter of the diagonal blocks cost ~120us of ring time at
    48-54 GB/s due to sub-1KiB descriptors).  Two DMAs with AP stride
    S+1, 1 KiB descriptors.
  - bottom-right 256x256 corner (rows 3840+, where the band would
    overrun the row end): one 256 KiB gather -> gpsimd affine_select
    (keep x at/below diagonal, 1.0 above) -> one scatter.

DMAs are spread over the three descriptor paths (SP ring via sync, ACT
ring via scalar, SWDGE via gpsimd) balancing estimated ring occupancy.
"""

import glob
import os
import tempfile

import numpy as np

from concourse import bass, mybir

S = 4096
P = 128
N_CORES = 8

_cached_nc = None


def _build():
    global _cached_nc
    if _cached_nc is not None:
        return _cached_nc

    nc = bass.Bass()
    out = nc.dram_tensor("out", [S, S], mybir.dt.float32, kind="ExternalOutput")

    # ones tile: 128 x 4096 f32 (2 MiB) -- every DMA below sources at most
    # 512K elements from it (element order is irrelevant: all are 1.0).
    F = 4096

    with (
        nc.Block() as block,
        nc.semaphore("dsem") as dsem,  # bulk ones/band DMA completions
        nc.semaphore("gsem") as gsem,  # corner gather done
        nc.semaphore("ssem") as ssem,  # corner scatter done
        nc.semaphore("msem") as msem,  # ones memset done
        nc.sbuf_tensor("ones", [P, F], mybir.dt.float32) as ones,
        nc.sbuf_tensor("corner_in", [P, P], mybir.dt.float32) as corner_in,
        nc.sbuf_tensor("corner_out", [P, P], mybir.dt.float32) as corner_out,
    ):
        # Quad-tree staircase level (width w, 2048//w rects, rect k at rows
        # [2wk, 2wk+w) x cols [2wk+w, 2wk+2w)), restricted to rects
        # [k0, k0+c).  All DMAs source <= 512K elements of the ones tile.
        def ones_level(eng, w, k0, c):
            return eng.dma_start(
                out=bass.AP(
                    out, w + k0 * 2 * w * (S + 1), [[2 * w * (S + 1), c], [S, w], [1, w]]
                ),
                in_=ones[:, : c * w * w // P],
            ).then_inc(dsem, 16)

        # 512-row slice of the 2048-level rect (rows[r0:r0+512] x
        # cols[2048:4096] halved into 256-row pieces for the 512K source
        # cap); 8 KiB descriptors.
        def ones_2048_quarter(eng, r0):
            for r in (r0, r0 + 256):
                eng.dma_start(
                    out=bass.AP(out, r * S + 2048, [[S, 256], [1, 2048]]),
                    in_=ones[:, :F],
                ).then_inc(dsem, 16)

        # Half (512 rows) of a 1024-level rect; 4 KiB descriptors.
        def ones_1024_half(eng, k, r0):
            eng.dma_start(
                out=bass.AP(
                    out, 1024 + k * 2048 * (S + 1) + r0 * S, [[S, 512], [1, 1024]]
                ),
                in_=ones[:, :F],
            ).then_inc(dsem, 16)

        # Corner: the last 128x128 diagonal block, rows/cols [3968:4096).
        corner_dram = bass.AP(out, 3968 * S + 3968, [[S, P], [1, P]])

        @block.vector
        def _(vector: bass.BassVectorEngine):
            vector.memset(ones[:, :], 1.0).then_inc(msem, 1)

        # Shifted-band write: out[r, r+1 : r+1+w) = 1 for nrows rows from
        # r0, covering the near-diagonal triangles; right of the diagonal
        # everything is 1, so spilling into the staircase region is a
        # harmless same-value overlap.
        def band(eng, r0, nrows, w):
            eng.dma_start(
                out=bass.AP(out, r0 * (S + 1) + 1, [[S + 1, nrows], [1, w]]),
                in_=ones[:, : nrows * w // P],
            ).then_inc(dsem, 16)

        @block.sync
        def _(sync: bass.BassEngine):
            sync.wait_ge(msem, 1)
            ones_1024_half(sync, 0, 0)
            ones_1024_half(sync, 0, 512)
            ones_2048_quarter(sync, 0)
            ones_2048_quarter(sync, 512)
            ones_level(sync, 512, 0, 1)
            ones_level(sync, 512, 1, 1)
            sync.wait_ge(dsem, 16 * 21)
            sync.wait_ge(ssem, 16)

        @block.scalar
        def _(scalar: bass.BassEngine):
            scalar.wait_ge(msem, 1)
            ones_1024_half(scalar, 1, 0)
            ones_1024_half(scalar, 1, 512)
            ones_2048_quarter(scalar, 1024)
            ones_2048_quarter(scalar, 1536)
            ones_level(scalar, 512, 3, 1)
            ones_level(scalar, 512, 2, 1)

        @block.gpsimd
        def _(gpsimd: bass.BassGpSimd):
            # Corner gather first: nothing else touches rows/cols 3968+.
            gpsimd.dma_start(out=corner_in[:, :], in_=corner_dram).then_inc(gsem, 16)
            gpsimd.wait_ge(msem, 1)
            # Sub-2KiB-descriptor work lives on SWDGE: it packs small
            # descriptors into 4 KiB packets (HWDGE does not).  Tiny
            # bookkeeping packets scale with descriptor count, and queues
            # share SDMA packet slots about equally, so the SWDGE queue
            # carries as few descriptors as possible: a 256-wide band
            # (which also replaces the 128-level for rows < 3840).
            ones_level(gpsimd, 256, 0, 8)
            ones_level(gpsimd, 128, 15, 1)
            band(gpsimd, 0, 1920, 256)
            band(gpsimd, 1920, 1920, 256)
            band(gpsimd, 3840, P, P)
            gpsimd.wait_ge(gsem, 16)
            # iota(p, c) = p - c; keep x where >= 0 (at/below diagonal).
            gpsimd.affine_select(
                out=corner_out[:, :],
                in_=corner_in[:, :],
                pattern=[[-1, P]],
                base=0,
                channel_multiplier=1,
                compare_op=mybir.AluOpType.is_ge,
                fill=1.0,
            )
            gpsimd.dma_start(out=corner_dram, in_=corner_out[:, :]).then_inc(ssem, 16)

    _cached_nc = nc
    return nc


def _sharded_fn(nc):
    """Build the 8-core PJRT launcher with the output buffer donated.

    Mirrors concourse.bass2jax.run_bass_via_pjrt's multi-core path, except
    the donated output staging buffer is caller-provided (we stage x, the
    in-place masked_fill source) instead of zeros.
    """
    import jax
    from concourse import bass2jax as b2j

    b2j.install_neuronx_cc_hook()

    partition_name = nc.partition_id_tensor.name if nc.partition_id_tensor else None
    in_names: list = []
    out_names: list = []
    out_avals: list = []
    for alloc in nc.m.functions[0].allocations:
        if not isinstance(alloc, mybir.MemoryLocationSet):
            continue
        name = alloc.memorylocations[0].name
        if alloc.kind == "ExternalInput":
            if name != partition_name:
                in_names.append(name)
        elif alloc.kind == "ExternalOutput":
            assert alloc.tensor_shape is not None and alloc.dtype is not None
            out_names.append(name)
            out_avals.append(
                jax.core.ShapedArray(tuple(alloc.tensor_shape), mybir.dt.np(alloc.dtype))
            )
    assert in_names == [] and out_names == ["out"], (in_names, out_names)
    all_in_names = tuple(in_names + out_names + ([partition_name] if partition_name else []))

    def _body(out_buf):
        operands = [out_buf]
        if partition_name is not None:
            operands.append(b2j.partition_id_tensor())
        outs = b2j._bass_exec_p.bind(
            *operands,
            out_avals=tuple(out_avals),
            in_names=all_in_names,
            out_names=tuple(out_names),
            lowering_input_output_aliases=(),
            sim_require_finite=True,
            sim_require_nnan=True,
            nc=nc,
        )
        return tuple(outs)

    devices = jax.devices()[:N_CORES]
    assert len(devices) == N_CORES, f"need {N_CORES} devices, got {len(devices)}"
    mesh = b2j.Mesh(np.asarray(devices), ("core",))
    spec = (b2j.PartitionSpec("core"),)
    return jax.jit(
        b2j.shard_map(_body, mesh=mesh, in_specs=spec, out_specs=spec, check_rep=False),
        donate_argnums=(0,),
        keep_unused=True,
    )


_cached_fn = None


def _run(x_full: np.ndarray, trace: bool = False):
    global _cached_fn
    nc = _build()
    if _cached_fn is None:
        _cached_fn = _sharded_fn(nc)

    x_full = np.ascontiguousarray(np.asarray(x_full, dtype=np.float32))
    staged = x_full.reshape(N_CORES * S, S)

    if not trace:
        out = _cached_fn(staged)[0]
        return np.asarray(out).reshape(N_CORES, S, S), None

    # Profiling path (test harness only): capture core 0's NTFF via the
    # axon hook and run the stock NTFF -> perfetto pipeline.
    from antenv.axon_hooks import get_axon_ntff_profile_hook
    import gauge.profiler
    from concourse import bass_utils
    from concourse._compat import FishPath

    hook = get_axon_ntff_profile_hook()
    neff_dir = tempfile.mkdtemp()
    with hook(neff_dir, [0]):
        out = _cached_fn(staged)[0]
    result = np.asarray(out).reshape(N_CORES, S, S)

    if not glob.glob(os.path.join(neff_dir, "*_body*.ntff")):
        return result, bass_utils.BassKernelResults(
            results=[], instructions_and_trace=None, profile_json=None,
            exec_time_ns=None,
        )
    sharepath = bass_utils.upload_artifacts(neff_dir)
    profile = gauge.profiler.Profile(
        profile_path=FishPath(neff_dir),
        kernel_dev_mode=True,
        profile_on_exit=False,
        bass_kernel=nc.m,
        offline_processing=True,
        fname="*_body*",
        metadata={"artifacts_path": sharepath},
    )
    res = bass_utils._process_ntff_profile(
        profile, neff_dir, nc, list(range(N_CORES)), [0], False, {}, False
    )
    return result, res.as_bass_kernel_results([])


def kernel(x: np.ndarray) -> np.ndarray:
    out, _ = _run(x, trace=False)
    return out
